# revision 64
# baseline (speedup 1.0000x reference)
"""Trainium2 Bass kernel for nn_MixAttention — v2 (batch x head-group sharding).

Sharding: 8 cores = 4 batches x 2 head-groups. Core (b, g) computes Q/K/V
projections (fp8e4 DoubleRow matmuls, 2x rate) for heads 4g..4g+3 over all
2048 rows of batch b, dual-stream attention for those 4 heads (scores bf16,
exp split across ACT engine and a DVE Schraudolph affine written directly
as fp8e5 uint8 bit patterns, PV in fp8 DoubleRow), then an output-projection
partial over all 2048 rows from its 4 heads (bf16). A pairwise
ReduceScatter(add) sums
the two head-groups' partials and scatters row-halves, after which each core
does residual + layernorm on its own 1024 rows. No per-core code divergence:
the scatter order (group rank -> row half) matches the host's unshard map.

Layout tricks:
 - Q/K weight columns are pair-packed with the t-stream pair SWAPPED so both
   partition halves of every projection PSUM evacuate partition-aligned into
   per-head [kd;kt]/[kt;kd] cat tiles (no DMA shuffles; scores are invariant
   to the within-head cat order as long as q and k agree).
 - V is projected with stationary xT (output lands t-major, no transposes),
   weights split hi+lo fp8 to kill correlated quantization error, written
   16x-scaled to fp8e4 with the softmax-ones column set to 16 (cancels).
 - Per-head parity places the ones/v columns so PV output rows never need a
   partition-shifted copy: even heads own psum rows 0:64 (sum row 64), odd
   heads rows 64:128 (sum row 63).
"""
import sys
import os

sys.path.insert(0, "/opt/trn_rl_repo")

import numpy as np
import ml_dtypes

import concourse.bass as bass
import concourse.mybir as mybir
import concourse.tile as tile
from concourse import bacc
from concourse import bass_utils

B, S, D = 4, 2048, 512
H, DH = 8, 64
HL = 4          # heads per core
SQ = S // 2     # output rows per core
EPS = 1e-5

F32 = mybir.dt.float32
BF = mybir.dt.bfloat16
F8E4 = mybir.dt.float8e4
F8E5 = mybir.dt.float8e5
U8 = mybir.dt.uint8
BF_NP = ml_dtypes.bfloat16
F8E4_NP = ml_dtypes.float8_e4m3
F8E5_NP = ml_dtypes.float8_e5m2

# tuning knobs (env-overridable for bringup)

V_E4 = os.environ.get("VE4", "1") == "1"          # vsum fp8e4 x16 vs e5m2
V_HILO = os.environ.get("VHILO", "1") == "1"      # hi+lo fp8 V weights
# Schraudolph-direct-to-e5m2: DVE writes uint8 bit patterns
#   bits = s * (0.5/ln2) + B8  (RN convert, saturates <0 to 0)
# which IS e5m2(exp(s/8) * 2^((B8-60)/4)). The ACT path matches via
#   C_OFF = (60-B8)*ln2/4  ->  exp(s/8 - C_OFF) = exp(s/8)*2^((B8-60)/4).
# B8=58: top bits ~121 (<123.5 inf boundary), scores <= -80 flush to +0.
B8 = float(os.environ.get("B8", "58.0"))
C_OFF = (60.0 - B8) * np.log(2) / 4.0
SCHR8_A = 0.5 / np.log(2)   # bits per raw-score unit: (4/ln2) * (1/8)

_MODULES = {}

RG_PAIRS = [[0, 1], [2, 3], [4, 5], [6, 7]]


def _build_module(reps=1, phases="all"):
    nc = bacc.Bacc("TRN2", target_bir_lowering=False, debug=False)

    d_x = {}
    for xn in ("xqd", "xqt", "xkd", "xkt", "xvd", "xvt"):
        d_x[xn] = nc.dram_tensor(xn, [D, S], F8E4, kind="ExternalInput")
    d_wqk = nc.dram_tensor("wqk", [8, 128, 2, 2, 128], F8E4, kind="ExternalInput")
    d_wv = nc.dram_tensor("wv", [4 if V_HILO else 2, 128, 2, 2, 256], F8E4,
                          kind="ExternalInput")
    d_bqk = nc.dram_tensor("bqk", [128, 8], F32, kind="ExternalInput")
    d_bv = nc.dram_tensor("bv", [1, 256], F32, kind="ExternalInput")
    d_wo = nc.dram_tensor("wo", [2, 128, D], BF, kind="ExternalInput")
    d_qres = nc.dram_tensor("qres", [SQ, D], BF, kind="ExternalInput")
    d_bo = nc.dram_tensor("bo", [1, D], F32, kind="ExternalInput")

    d_out = nc.dram_tensor("out", [SQ, D], F32, kind="ExternalOutput")

    with tile.TileContext(nc) as tc:
        import contextlib

        with contextlib.ExitStack() as top:
            if reps > 1:
                top.enter_context(tc.For_i(0, reps, 1))
            # Collectives cannot replay inside a hardware loop (mesh
            # desyncs), so timing builds (reps>1) skip the ReduceScatter;
            # its cost is measured separately via RSX extra collectives.
            _emit_body(nc, tc, top, d_x, d_wqk, d_wv, d_bqk, d_bv, d_wo,
                       d_qres, d_bo, d_out,
                       do_rs=(reps == 1), phases=phases)

    nc.compile()
    return nc


def _emit_body(nc, tc, top, d_x, d_wqk, d_wv, d_bqk, d_bv, d_wo,
               d_qres, d_bo, d_out, do_rs=True,
               phases="all"):
    import contextlib
    Act = mybir.ActivationFunctionType
    Alu = mybir.AluOpType
    Ax = mybir.AxisListType
    DR = mybir.MatmulPerfMode.DoubleRow
    F8V = F8E4 if V_E4 else F8E5
    AT_DT = F8E5
    VSC = 1.0 if V_E4 else (1.0 / 16.0)
    ONEVAL = 16.0 if V_E4 else 1.0

    consts = top.enter_context(tc.tile_pool(name="consts", bufs=1))
    resid = top.enter_context(tc.tile_pool(name="resid", bufs=1))

    # ---- constants ------------------------------------------------------
    # Queue split: scalar HWDGE gets the weights needed first (wqk/bqk) and
    # the t-stream activations; sync HWDGE streams the d-stream activations
    # from t=0 plus the late-needed wv/wo. gamma/beta are identity in this
    # problem and applied host-side if ever nontrivial.
    wqk_sb = consts.tile([128, 8, 2, 2, 128], F8E4, tag="wqk")
    wv_sb = consts.tile([128, 4 if V_HILO else 2, 2, 2, 256], F8E4, tag="wv")
    wo_sb = consts.tile([128, 2, D], BF, tag="wo")
    bqk_sb = consts.tile([128, 8], F32, tag="bqk")
    nc.scalar.dma_start(wqk_sb[:],
                        d_wqk.ap().rearrange("t p j i m -> p t j i m"))
    nc.scalar.dma_start(bqk_sb[:], d_bqk.ap())

    bv1 = consts.tile([1, 256], F32, tag="bv1")
    bo1 = consts.tile([1, D], F32, tag="bo1")
    bvB = consts.tile([128, 256], F32, tag="bvB")
    boB = consts.tile([128, D], F32, tag="boB")
    negC = consts.tile([128, 1], F32, tag="negC")
    nc.gpsimd.memset(negC[:], -C_OFF)
    epsC = consts.tile([128, 1], F32, tag="epsC")
    nc.gpsimd.memset(epsC[:], EPS)
    mhalfC = consts.tile([128, 1], F32, tag="mhalfC")
    nc.gpsimd.memset(mhalfC[:], -0.5)

    # ---- resident activations -------------------------------------------
    resb = resid.tile([128, 8, D], F32, tag="resb")
    kcat = [resid.tile([128, S], BF, tag=f"kcat{h}", name=f"kcat{h}")
            for h in range(HL)]
    qcat = [resid.tile([128, S], BF, tag=f"qcat{h}", name=f"qcat{h}")
            for h in range(HL)]
    vsum = resid.tile([128, 16, HL, 68], F8V, tag="vsum")
    ctp = [resid.tile([128, S], BF, tag=f"ctp{p}", name=f"ctp{p}")
           for p in range(2)]
    nc.gpsimd.memset(vsum[:, :, :, 64:65], ONEVAL)  # softmax-sum ones column
    nc.gpsimd.memset(vsum[:, :, :, 65:68], 0.0)     # pad (dual-fp8 align)

    # ---- Phase A: projections (fp8 DoubleRow) ---------------------------
    with (
        tc.tile_pool(name="xt", bufs=1) as xtp,
        tc.tile_pool(name="qk_ps", bufs=2, space="PSUM") as qkps,
        tc.tile_pool(name="v_ps", bufs=4, space="PSUM") as vps,
    ):
        xt = {}
        for n_i, xn in enumerate(("xkd", "xkt", "xqd", "xqt", "xvd", "xvt")):
            xt[xn] = xtp.tile([128, 4, S], F8E4, tag=f"xt_{xn}", name=f"xt_{xn}")
            eng = nc.sync if n_i % 2 == 0 else nc.scalar
            eng.dma_start(
                xt[xn][:], d_x[xn].ap().rearrange("(kc p) s -> p kc s", p=128))
        # late-needed constants queued behind the critical activations
        nc.sync.dma_start(wv_sb[:],
                          d_wv.ap().rearrange("t p j i m -> p t j i m"))
        nc.sync.dma_start(wo_sb[:], d_wo.ap().rearrange("t p m -> p t m"))
        nc.scalar.dma_start(bv1[:], d_bv.ap())
        nc.scalar.dma_start(bo1[:], d_bo.ap())
        nc.gpsimd.partition_broadcast(bvB[:], bv1[:])
        nc.gpsimd.partition_broadcast(boB[:], bo1[:])

        # residual + bo staging (gpsimd SWDGE casting DMA bf16->f32; keeps
        # both HWDGE queues free for the critical xt/weight loads)
        for st in range(8):
            qr = resid.tile([128, D], F32, tag="qr", name="qr", bufs=2)
            nc.gpsimd.dma_start(qr[:], d_qres.ap()[st * 128:(st + 1) * 128, :])
            nc.gpsimd.tensor_add(resb[:, st, :], qr[:], boB[:])

        # Q/K: tensors (idx, xname, dest, swap): d-stream natural, t-stream
        # pair-swapped so all evac copies are partition-aligned. Chunks run
        # in pairs on two rotating PSUM tiles with j outermost so the two
        # matmuls sharing a weight pair are adjacent (one LDWEIGHTS each).
        for t_idx, xn, dest, swap in (
            (2, "xkd", kcat, False), (3, "xkt", kcat, True),
            (0, "xqd", qcat, False), (1, "xqt", qcat, True),
        ):
            for P in range(2):
                col = t_idx * 2 + P
                for chp in range(2):
                    pss = [(2 * chp, qkps.tile([128, 512], F32, tag="qkpA",
                                               name="qkpA")),
                           (2 * chp + 1, qkps.tile([128, 512], F32,
                                                   tag="qkpB", name="qkpB"))]
                    for j in range(2):
                        for cc, ps in pss:
                            nc.tensor.matmul(
                                ps[:], lhsT=wqk_sb[:, col, j, :, :],
                                rhs=xt[xn][:, 2 * j:2 * j + 2,
                                           cc * 512:(cc + 1) * 512],
                                start=(j == 0), stop=(j == 1), perf_mode=DR)
                    h_lo = 2 * P + (1 if swap else 0)
                    h_hi = 2 * P + (0 if swap else 1)
                    for cc, ps in pss:
                        cs = slice(cc * 512, (cc + 1) * 512)
                        if cc % 2 == 0:
                            nc.vector.tensor_scalar(
                                dest[h_lo][0:64, cs], ps[0:64, :],
                                1.0 / 16.0, bqk_sb[0:64, col:col + 1],
                                op0=Alu.mult, op1=Alu.add)
                            nc.vector.tensor_scalar(
                                dest[h_hi][64:128, cs], ps[64:128, :],
                                1.0 / 16.0, bqk_sb[64:128, col:col + 1],
                                op0=Alu.mult, op1=Alu.add)
                        else:
                            nc.scalar.activation(
                                dest[h_lo][0:64, cs], ps[0:64, :],
                                Act.Identity,
                                bias=bqk_sb[0:64, col:col + 1],
                                scale=1.0 / 16.0)
                            nc.scalar.activation(
                                dest[h_hi][64:128, cs], ps[64:128, :],
                                Act.Identity,
                                bias=bqk_sb[64:128, col:col + 1],
                                scale=1.0 / 16.0)

        # V: stationary xT (t-major psum), hi+lo weights, summed streams.
        # Loop order keeps matmuls sharing an xT stationary slice adjacent
        # (hi+lo pairs) so their LDWEIGHTS dedupe.
        bv_hd = bvB[:, :].rearrange("p (h c) -> p h c", c=64)
        for sc_i in range(16):
            ps = vps.tile([128, 256], F32, tag="vp", name="vp")
            if V_HILO:
                seq = [(0, "xvd", 0), (1, "xvd", 0), (0, "xvd", 1),
                       (1, "xvd", 1), (2, "xvt", 0), (3, "xvt", 0),
                       (2, "xvt", 1), (3, "xvt", 1)]
            else:
                seq = [(0, "xvd", 0), (0, "xvd", 1),
                       (1, "xvt", 0), (1, "xvt", 1)]
            nk = len(seq)
            for k, (widx, xn, j) in enumerate(seq):
                nc.tensor.matmul(
                    ps[:],
                    lhsT=xt[xn][:, 2 * j:2 * j + 2,
                                sc_i * 128:(sc_i + 1) * 128],
                    rhs=wv_sb[:, widx, j, :, :],
                    start=(k == 0), stop=(k == nk - 1), perf_mode=DR)
            psv = ps[:, :].rearrange("p (h c) -> p h c", c=64)
            nc.vector.scalar_tensor_tensor(
                out=vsum[:, sc_i, :, 0:64], in0=psv[:], scalar=VSC,
                in1=bv_hd[:], op0=Alu.mult, op1=Alu.add)

    if phases == "proj":
        nc.sync.dma_start(d_out.ap()[0:128, :], resb[:, 0, :])
        return
    # ---- Phase B: attention (software-pipelined) ------------------------
    # Per 128-key chunk: 2 score matmuls -> exp split across ACT (query
    # cols 0:512, exact exp to f8e5) and DVE (cols 512:1024, Schraudolph
    # affine written straight into a second f8e5 tile as uint8 bits -- no
    # convert pass, and no shared-tile write dep between the two engines).
    # PV runs two chunk-pairs behind the scores. Softmax-sum reciprocal is
    # DMA-reshaped [1,1024]->[128,8] so the DVE divide uses all lanes.
    with (
        tc.tile_pool(name="sc_ps", bufs=2, space="PSUM") as scp,
        tc.tile_pool(name="ctx_ps", bufs=2, space="PSUM") as ctxp,
        tc.tile_pool(name="at", bufs=3) as atp,
        tc.tile_pool(name="rin", bufs=2) as rip,
        tc.tile_pool(name="rb", bufs=2) as rbp,
        tc.tile_pool(name="ctmp", bufs=2) as ctmp,
    ):
        def emit_pv(ctx_ps, h, at2, c2):
            # NOTE: start=True zeroes the whole 2KB PSUM bank, so only
            # the first chunk touching each bank may start.
            for o in range(2):
                nc.tensor.matmul(
                    ctx_ps[0:68, o * 512:(o + 1) * 512],
                    lhsT=vsum[:, 2 * c2:2 * c2 + 2, h, 0:68],
                    rhs=at2[:, :, o * 512:(o + 1) * 512],
                    start=(c2 == 0), stop=(c2 == 7), perf_mode=DR)

        # evac: divide rows 0:64 by sum row 64 -> ctp pair tile. Staged so
        # each op is EMITTED into its engine queue a few chunks into the
        # NEXT (h,qh) stream -- by then its deps are met and the strict
        # FIFO engine queues don't stall the per-chunk exp work behind it.
        # Even heads land partition-aligned; odd heads go through a temp
        # tile + partition-shifted SBUF DMA (engines can't shift).
        def evac_stage1(ctx_ps, h, qh):
            rin = rip.tile([1, 1024], F32, tag="rin", name="rin")
            nc.vector.tensor_copy(rin[:], ctx_ps[64:65, :])
            r128 = rip.tile([128, 8], F32, tag="r128", name="r128")
            nc.sync.dma_start(r128[:], rin[:])
            r128b = rip.tile([128, 8], F32, tag="r128b", name="r128b")
            nc.vector.reciprocal(r128b[:], r128[:])
            rin2 = rip.tile([1, 1024], F32, tag="rin2", name="rin2")
            nc.sync.dma_start(rin2[:], r128b[:])
            return rin2

        def evac_stage2(ctx_ps, h, qh, rin2):
            par, P = h % 2, h // 2
            rb = rbp.tile([64, 1024], F32, tag="rb", name="rb")
            nc.gpsimd.partition_broadcast(rb[:], rin2[:])
            if par == 0:
                nc.vector.tensor_mul(
                    ctp[P][0:64, qh * 1024:(qh + 1) * 1024],
                    ctx_ps[0:64, :], rb[:])
            else:
                ct = ctmp.tile([64, 1024], BF, tag="ct", name="ct")
                nc.vector.tensor_mul(ct[:], ctx_ps[0:64, :], rb[:])
                nc.sync.dma_start(
                    ctp[P][64:128, qh * 1024:(qh + 1) * 1024], ct[:])

        pend_evac = None   # (ctx_ps, h, qh, rin2_or_None)
        for h in range(HL):
            for qh in range(2):
                ctx_ps = ctxp.tile([128, 1024], F32, tag="ctx", name="ctx")
                pendq = []   # PV runs TWO chunk-pairs behind the scores so
                # it never waits on the exp writes of the at2 it consumes
                for c2 in range(8):
                    # exp engine alternates by c2: even c2 -> ACT (exact
                    # exp), odd c2 -> DVE (Schraudolph-to-u8). One full
                    # [128,1024] op per key-chunk halves the per-op
                    # dispatch/drain overhead vs column-splitting, and
                    # each c2's chunk pair stays in one tile so the PV
                    # DoubleRow pairing is unchanged.
                    use_act = (c2 % 2 == 0)
                    at2 = atp.tile([128, 2, 1024], AT_DT,
                                   tag="atA" if use_act else "atB",
                                   name="atA" if use_act else "atB")
                    for i in range(2):
                        tcn = 2 * c2 + i
                        sc = scp.tile([128, 1024], F32, tag="sc", name="sc")
                        for half in range(2):
                            nc.tensor.matmul(
                                sc[:, half * 512:(half + 1) * 512],
                                lhsT=kcat[h][:, tcn * 128:(tcn + 1) * 128],
                                rhs=qcat[h][:, qh * 1024 + half * 512:
                                            qh * 1024 + (half + 1) * 512],
                                start=True, stop=True)
                        if use_act:
                            nc.scalar.activation(
                                at2[:, i, :], sc[:], Act.Exp,
                                bias=negC[:, 0:1], scale=0.125)
                        else:
                            nc.vector.tensor_scalar(
                                at2[:, i, :].bitcast(U8), sc[:],
                                SCHR8_A, B8, op0=Alu.mult, op1=Alu.add)
                    pendq.append((ctx_ps, h, at2, c2))
                    if len(pendq) == 3:
                        emit_pv(*pendq.pop(0))
                    if c2 == 1 and pend_evac is not None:
                        pend_evac = pend_evac[:3] + (
                            evac_stage1(*pend_evac[:3]),)
                    elif c2 == 4 and pend_evac is not None:
                        evac_stage2(*pend_evac)
                        pend_evac = None
                for pq in pendq:
                    emit_pv(*pq)
                pendq = []
                pend_evac = (ctx_ps, h, qh, None)
        pend_evac = pend_evac[:3] + (evac_stage1(*pend_evac[:3]),)
        evac_stage2(*pend_evac)

    if phases == "projattn":
        nc.sync.dma_start(d_out.ap()[0:128, :], resb[:, 0, :])
        return
    # ---- Phase C: out-proj partial + ReduceScatter + layernorm ----------
    with (
        tc.tile_pool(name="dram", bufs=1, space="DRAM") as dram,
        tc.tile_pool(name="o_ps", bufs=2, space="PSUM") as ops,
        tc.tile_pool(name="ob", bufs=3) as obp,
        tc.tile_pool(name="xs", bufs=2) as xsp,
        tc.tile_pool(name="ss", bufs=2) as ssp,
    ):
        snd = dram.tile([S, D], BF, tag="snd")
        rcv = dram.tile([SQ, D], BF, tag="rcv")
        for st in range(16):
            po = ops.tile([128, D], F32, tag="po", name="po")
            for P in range(2):
                nc.tensor.matmul(
                    po[:], lhsT=ctp[P][:, st * 128:(st + 1) * 128],
                    rhs=wo_sb[:, P, :], start=(P == 0), stop=(P == 1))
            ob = obp.tile([128, D], BF, tag="ob", name="ob")
            # copies alternate DVE/ACT; all snd triggers on sync so the
            # scalar queue isn't the Phase C pacer (copies + triggers)
            if st % 2 == 0:
                nc.vector.tensor_copy(ob[:], po[:])
            else:
                nc.scalar.copy(ob[:], po[:])
            nc.sync.dma_start(snd[st * 128:(st + 1) * 128, :], ob[:])
        if do_rs:
            nc.gpsimd.collective_compute(
                "ReduceScatter", Alu.add, replica_groups=RG_PAIRS,
                ins=[snd.opt()], outs=[rcv.opt()])
            rsx = int(os.environ.get("RSX", "0"))
            if rsx:
                dum = [dram.tile([SQ, D], BF, tag=f"dum{i}", name=f"dum{i}")
                       for i in range(2)]
                for i in range(rsx):
                    nc.gpsimd.collective_compute(
                        "ReduceScatter", Alu.add, replica_groups=RG_PAIRS,
                        ins=[snd.opt()], outs=[dum[i % 2].opt()])
        # layernorm: two passes. Pass 1 accumulates x = rcv + residual
        # in-place over resb and collects bn-stats; a single batched
        # Ln+Exp pair (one table set switch each, not per-st) computes
        # rs = (var+eps)^-0.5 for all 8 st at once; pass 2 is one fused
        # (x*rs - mu*rs) op per st. gamma/beta are identity here (host
        # applies them if ever nontrivial).
        # tiny dead matmuls spaced through the LN tail keep the PE's HAM
        # activity window non-idle, so the next iteration's matmuls start
        # at full clock instead of re-warming from 1.2 GHz. Each reads the
        # st's LN data so the RAW dep paces it to the LN pipeline.
        def warm_pe(dep_ap):
            wm = ops.tile([64, 64], F32, tag="warm", name="warm")
            nc.tensor.matmul(wm[:], lhsT=dep_ap[0:128, 0:64],
                             rhs=dep_ap[0:128, 64:128], start=True,
                             stop=True)

        mv8 = ssp.tile([128, 8, 2], F32, tag="mv8", name="mv8")
        for st in range(8):
            xr = obp.tile([128, D], BF, tag="xr", name="xr")
            eng = nc.sync if st % 2 == 0 else nc.scalar
            eng.dma_start(xr[:], rcv[st * 128:(st + 1) * 128, :])
            nc.vector.tensor_add(resb[:, st, :], xr[:], resb[:, st, :])
            bst = ssp.tile([128, 6], F32, tag="bst", name="bst")
            nc.vector.bn_stats(bst[:], resb[:, st, :])
            nc.vector.bn_aggr(mv8[:, st, :], bst[:])
            warm_pe(resb[:, st, :])
        lnv8 = ssp.tile([128, 8], F32, tag="lnv8", name="lnv8")
        nc.scalar.activation(lnv8[:], mv8[:, :, 1], Act.Ln, bias=epsC[:, 0:1])
        rs8 = ssp.tile([128, 8], F32, tag="rs8", name="rs8")
        nc.scalar.activation(rs8[:], lnv8[:], Act.Exp, scale=mhalfC[:, 0:1])
        nmr8 = ssp.tile([128, 8], F32, tag="nmr8", name="nmr8")
        nc.vector.scalar_tensor_tensor(
            out=nmr8[:], in0=mv8[:, :, 0], scalar=-1.0, in1=rs8[:],
            op0=Alu.mult, op1=Alu.mult)
        for st in range(8):
            y = xsp.tile([128, D], F32, tag="y", name="y")
            nc.vector.tensor_scalar(
                y[:], resb[:, st, :], rs8[:, st:st + 1],
                nmr8[:, st:st + 1], op0=Alu.mult, op1=Alu.add)
            eng = nc.sync if st % 2 == 1 else nc.scalar
            eng.dma_start(d_out.ap()[st * 128:(st + 1) * 128, :], y[:])
            warm_pe(y[:])


def get_module(reps=1):
    phases = os.environ.get("KPHASES", "all")
    key = (reps, phases)
    if key not in _MODULES:
        _MODULES[key] = _build_module(reps, phases)
    return _MODULES[key]


def make_in_maps(inputs):
    """Build the 8 per-core input maps from the full problem inputs."""
    xT = {}
    for key in ("Q_data", "Q_time", "K_data", "K_time", "V_data", "V_time"):
        xT[key] = [np.ascontiguousarray(
            inputs[key][bb].astype(np.float32).T).astype(F8E4_NP)
            for bb in range(B)]

    F8V_NP = F8E4_NP if V_E4 else F8E5_NP
    VBS = 16.0 if V_E4 else 1.0

    def wq_tile(Wfull, heads, swap):
        # [512, 128] cols = [head a | head b] (b,a if swap); -> [128,2,2,128]
        a, b = heads
        if swap:
            a, b = b, a
        cols = np.concatenate(
            [Wfull[:, a * DH:(a + 1) * DH], Wfull[:, b * DH:(b + 1) * DH]], 1)
        w = (16.0 * cols).astype(F8E4_NP)
        return np.ascontiguousarray(
            w.reshape(2, 2, 128, 128).transpose(2, 0, 1, 3))

    def bq_col(bfull, heads, swap):
        a, b = heads
        if swap:
            a, b = b, a
        return np.concatenate(
            [bfull[a * DH:(a + 1) * DH], bfull[b * DH:(b + 1) * DH]]).astype(
                np.float32)

    in_maps = []
    for c in range(8):
        bb, g = divmod(c, 2)
        h0 = 4 * g
        m = {
            "xqd": xT["Q_data"][bb], "xqt": xT["Q_time"][bb],
            "xkd": xT["K_data"][bb], "xkt": xT["K_time"][bb],
            "xvd": xT["V_data"][bb], "xvt": xT["V_time"][bb],
        }
        # wqk [8, 128, 2, 2, 128]: idx = t*2 + P; t order qd,qt,kd,kt
        wqk = np.empty((8, 128, 2, 2, 128), F8E4_NP)
        bqk = np.empty((128, 8), np.float32)
        for t_i, (wkey, bkey, swap) in enumerate((
                ("Wq_d", "bq_d", False), ("Wq_t", "bq_t", True),
                ("Wk_d", "bk_d", False), ("Wk_t", "bk_t", True))):
            W = np.asarray(inputs[wkey], np.float32)
            bias = np.asarray(inputs[bkey], np.float32)
            for P in range(2):
                heads = (h0 + 2 * P, h0 + 2 * P + 1)
                wqk[t_i * 2 + P] = wq_tile(W, heads, swap)
                bqk[:, t_i * 2 + P] = bq_col(bias, heads, swap)
        m["wqk"] = wqk
        m["bqk"] = bqk
        # wv: vd_hi, vd_lo, vt_hi, vt_lo (or hi-only) in natural cols
        wv = np.empty((4 if V_HILO else 2, 128, 2, 2, 256), F8E4_NP)
        for s_i, wkey in enumerate(("Wv_d", "Wv_t")):
            W = 16.0 * np.asarray(
                inputs[wkey], np.float32)[:, h0 * DH:(h0 + 4) * DH]
            hi = W.astype(F8E4_NP)
            if V_HILO:
                lo = (W - hi.astype(np.float32)).astype(F8E4_NP)
                for k_i, Wp in enumerate((hi, lo)):
                    wv[s_i * 2 + k_i] = np.ascontiguousarray(
                        Wp.reshape(2, 2, 128, 256).transpose(2, 0, 1, 3))
            else:
                wv[s_i] = np.ascontiguousarray(
                    hi.reshape(2, 2, 128, 256).transpose(2, 0, 1, 3))
        m["wv"] = wv
        bv = (np.asarray(inputs["bv_d"], np.float32)
              + np.asarray(inputs["bv_t"], np.float32))[
                  h0 * DH:(h0 + 4) * DH] * VBS
        m["bv"] = bv.reshape(1, 256)
        # wo [2, 128, 512]: pair P rows = heads (h0+2P, h0+2P+1) dims
        wo = np.empty((2, 128, D), BF_NP)
        Wo = np.asarray(inputs["Wo"], np.float32)
        for P in range(2):
            r0 = (h0 + 2 * P) * DH
            wo[P] = Wo[r0:r0 + 128, :].astype(BF_NP)
        m["wo"] = wo
        m["qres"] = np.ascontiguousarray(
            inputs["Q_data"][bb, g * SQ:(g + 1) * SQ, :].astype(BF_NP))
        m["bo"] = np.asarray(inputs["bo"], np.float32).reshape(1, D)

        in_maps.append(m)
    return in_maps


def kernel(**inputs):
    inputs = {k: np.asarray(v) for k, v in inputs.items()}
    nc = get_module(reps=1)
    in_maps = make_in_maps(inputs)
    res = bass_utils.run_bass_kernel_spmd(nc, in_maps, core_ids=list(range(8)))
    out = np.empty((B, S, D), dtype=np.float32)
    for c in range(8):
        bb, g = divmod(c, 2)
        out[bb, g * SQ:(g + 1) * SQ, :] = res.results[c]["out"]
    # gamma/beta are identity in this problem; apply on host if not.
    gamma = np.asarray(inputs["gamma"], np.float32)
    beta = np.asarray(inputs["beta"], np.float32)
    if np.any(gamma != 1.0):
        out *= gamma
    if np.any(beta != 0.0):
        out += beta
    return out



# revision 65
# speedup vs baseline: 1.1666x; 1.1666x over previous
"""Trainium2 Bass kernel for nn_MixAttention — v2 (batch x head-group sharding).

Sharding: 8 cores = 4 batches x 2 head-groups. Core (b, g) computes Q/K/V
projections (fp8e4 DoubleRow matmuls, 2x rate) for heads 4g..4g+3 over all
2048 rows of batch b, dual-stream attention for those 4 heads (scores bf16,
exp split across ACT engine and a DVE Schraudolph affine written directly
as fp8e5 uint8 bit patterns, PV in fp8 DoubleRow), then an output-projection
partial over all 2048 rows from its 4 heads (bf16). A pairwise
ReduceScatter(add) sums
the two head-groups' partials and scatters row-halves, after which each core
does residual + layernorm on its own 1024 rows. No per-core code divergence:
the scatter order (group rank -> row half) matches the host's unshard map.

Layout tricks:
 - Q/K weight columns are pair-packed with the t-stream pair SWAPPED so both
   partition halves of every projection PSUM evacuate partition-aligned into
   per-head [kd;kt]/[kt;kd] cat tiles (no DMA shuffles; scores are invariant
   to the within-head cat order as long as q and k agree).
 - V is projected with stationary xT (output lands t-major, no transposes),
   weights split hi+lo fp8 to kill correlated quantization error, written
   16x-scaled to fp8e4 with the softmax-ones column set to 16 (cancels).
 - Per-head parity places the ones/v columns so PV output rows never need a
   partition-shifted copy: even heads own psum rows 0:64 (sum row 64), odd
   heads rows 64:128 (sum row 63).
"""
import sys
import os

sys.path.insert(0, "/opt/trn_rl_repo")

import numpy as np
import ml_dtypes

import concourse.bass as bass
import concourse.mybir as mybir
import concourse.tile as tile
from concourse import bacc
from concourse import bass_utils

B, S, D = 4, 2048, 512
H, DH = 8, 64
HL = 4          # heads per core
SQ = S // 2     # output rows per core
EPS = 1e-5

F32 = mybir.dt.float32
BF = mybir.dt.bfloat16
F8E4 = mybir.dt.float8e4
F8E5 = mybir.dt.float8e5
U8 = mybir.dt.uint8
BF_NP = ml_dtypes.bfloat16
F8E4_NP = ml_dtypes.float8_e4m3
F8E5_NP = ml_dtypes.float8_e5m2

# tuning knobs (env-overridable for bringup)

V_E4 = os.environ.get("VE4", "1") == "1"          # vsum fp8e4 x16 vs e5m2
V_HILO = os.environ.get("VHILO", "1") == "1"      # hi+lo fp8 V weights
# Schraudolph-direct-to-e5m2: DVE writes uint8 bit patterns
#   bits = s * (0.5/ln2) + B8  (RN convert, saturates <0 to 0)
# which IS e5m2(exp(s/8) * 2^((B8-60)/4)). The ACT path matches via
#   C_OFF = (60-B8)*ln2/4  ->  exp(s/8 - C_OFF) = exp(s/8)*2^((B8-60)/4).
# B8=58: top bits ~121 (<123.5 inf boundary), scores <= -80 flush to +0.
B8 = float(os.environ.get("B8", "58.0"))
C_OFF = (60.0 - B8) * np.log(2) / 4.0
SCHR8_A = 0.5 / np.log(2)   # bits per raw-score unit: (4/ln2) * (1/8)

_MODULES = {}

RG_PAIRS = [[0, 1], [2, 3], [4, 5], [6, 7]]


def _build_module(reps=1, phases="all"):
    nc = bacc.Bacc("TRN2", target_bir_lowering=False, debug=False)

    d_x = {}
    for xn in ("xqd", "xqt", "xkd", "xkt", "xvd", "xvt"):
        d_x[xn] = nc.dram_tensor(xn, [D, S], F8E4, kind="ExternalInput")
    d_wqk = nc.dram_tensor("wqk", [8, 128, 2, 2, 128], F8E4, kind="ExternalInput")
    d_wv = nc.dram_tensor("wv", [4 if V_HILO else 2, 128, 2, 2, 256], F8E4,
                          kind="ExternalInput")
    d_bqk = nc.dram_tensor("bqk", [128, 8], F32, kind="ExternalInput")
    d_bv = nc.dram_tensor("bv", [1, 256], F32, kind="ExternalInput")
    d_wo = nc.dram_tensor("wo", [2, 128, D], BF, kind="ExternalInput")
    d_qres = nc.dram_tensor("qres", [SQ, D], BF, kind="ExternalInput")
    d_bo = nc.dram_tensor("bo", [1, D], F32, kind="ExternalInput")

    d_out = nc.dram_tensor("out", [SQ, D], F32, kind="ExternalOutput")

    with tile.TileContext(nc) as tc:
        import contextlib

        with contextlib.ExitStack() as top:
            if reps > 1:
                top.enter_context(tc.For_i(0, reps, 1))
            # Collectives cannot replay inside a hardware loop (mesh
            # desyncs), so timing builds (reps>1) skip the ReduceScatter;
            # its cost is measured separately via RSX extra collectives.
            _emit_body(nc, tc, top, d_x, d_wqk, d_wv, d_bqk, d_bv, d_wo,
                       d_qres, d_bo, d_out,
                       do_rs=(reps == 1), phases=phases)

    nc.compile()
    return nc


def _emit_body(nc, tc, top, d_x, d_wqk, d_wv, d_bqk, d_bv, d_wo,
               d_qres, d_bo, d_out, do_rs=True,
               phases="all"):
    import contextlib
    Act = mybir.ActivationFunctionType
    Alu = mybir.AluOpType
    Ax = mybir.AxisListType
    DR = mybir.MatmulPerfMode.DoubleRow
    F8V = F8E4 if V_E4 else F8E5
    AT_DT = F8E5
    VSC = 1.0 if V_E4 else (1.0 / 16.0)
    ONEVAL = 16.0 if V_E4 else 1.0

    consts = top.enter_context(tc.tile_pool(name="consts", bufs=1))
    resid = top.enter_context(tc.tile_pool(name="resid", bufs=1))

    # ---- constants ------------------------------------------------------
    # Queue split: scalar HWDGE gets the weights needed first (wqk/bqk) and
    # the t-stream activations; sync HWDGE streams the d-stream activations
    # from t=0 plus the late-needed wv/wo. gamma/beta are identity in this
    # problem and applied host-side if ever nontrivial.
    wqk_sb = consts.tile([128, 8, 2, 2, 128], F8E4, tag="wqk")
    wv_sb = consts.tile([128, 4 if V_HILO else 2, 2, 2, 256], F8E4, tag="wv")
    wo_sb = consts.tile([128, 2, D], BF, tag="wo")
    bqk_sb = consts.tile([128, 8], F32, tag="bqk")
    nc.scalar.dma_start(wqk_sb[:],
                        d_wqk.ap().rearrange("t p j i m -> p t j i m"))
    nc.scalar.dma_start(bqk_sb[:], d_bqk.ap())

    bv1 = consts.tile([1, 256], F32, tag="bv1")
    bo1 = consts.tile([1, D], F32, tag="bo1")
    bvB = consts.tile([128, 256], F32, tag="bvB")
    boB = consts.tile([128, D], F32, tag="boB")
    negC = consts.tile([128, 1], F32, tag="negC")
    nc.gpsimd.memset(negC[:], -C_OFF)
    epsC = consts.tile([128, 1], F32, tag="epsC")
    nc.gpsimd.memset(epsC[:], EPS)
    mhalfC = consts.tile([128, 1], F32, tag="mhalfC")
    nc.gpsimd.memset(mhalfC[:], -0.5)

    # ---- resident activations -------------------------------------------
    resb = resid.tile([128, 8, D], F32, tag="resb")
    kcat = [resid.tile([128, S], BF, tag=f"kcat{h}", name=f"kcat{h}")
            for h in range(HL)]
    qcat = [resid.tile([128, S], BF, tag=f"qcat{h}", name=f"qcat{h}")
            for h in range(HL)]
    vsum = resid.tile([128, 16, HL, 68], F8V, tag="vsum")
    ctp = [resid.tile([128, S], BF, tag=f"ctp{p}", name=f"ctp{p}")
           for p in range(2)]
    nc.gpsimd.memset(vsum[:, :, :, 64:65], ONEVAL)  # softmax-sum ones column
    nc.gpsimd.memset(vsum[:, :, :, 65:68], 0.0)     # pad (dual-fp8 align)

    # ---- Phase A: projections (fp8 DoubleRow) ---------------------------
    with (
        tc.tile_pool(name="xt", bufs=1) as xtp,
        tc.tile_pool(name="qk_ps", bufs=2, space="PSUM") as qkps,
        tc.tile_pool(name="v_ps", bufs=4, space="PSUM") as vps,
    ):
        xt = {}
        for n_i, xn in enumerate(("xkd", "xkt", "xqd", "xqt", "xvd", "xvt")):
            xt[xn] = xtp.tile([128, 4, S], F8E4, tag=f"xt_{xn}", name=f"xt_{xn}")
            eng = nc.sync if n_i % 2 == 0 else nc.scalar
            eng.dma_start(
                xt[xn][:], d_x[xn].ap().rearrange("(kc p) s -> p kc s", p=128))
        # late-needed constants queued behind the critical activations
        nc.sync.dma_start(wv_sb[:],
                          d_wv.ap().rearrange("t p j i m -> p t j i m"))
        nc.sync.dma_start(wo_sb[:], d_wo.ap().rearrange("t p m -> p t m"))
        nc.scalar.dma_start(bv1[:], d_bv.ap())
        nc.scalar.dma_start(bo1[:], d_bo.ap())
        nc.gpsimd.partition_broadcast(bvB[:], bv1[:])
        nc.gpsimd.partition_broadcast(boB[:], bo1[:])

        # residual + bo staging (gpsimd SWDGE casting DMA bf16->f32; keeps
        # both HWDGE queues free for the critical xt/weight loads)
        for st in range(8):
            qr = resid.tile([128, D], F32, tag="qr", name="qr", bufs=2)
            nc.gpsimd.dma_start(qr[:], d_qres.ap()[st * 128:(st + 1) * 128, :])
            nc.gpsimd.tensor_add(resb[:, st, :], qr[:], boB[:])

        # Q/K: tensors (idx, xname, dest, swap): d-stream natural, t-stream
        # pair-swapped so all evac copies are partition-aligned. Chunks run
        # in pairs on two rotating PSUM tiles with j outermost so the two
        # matmuls sharing a weight pair are adjacent (one LDWEIGHTS each).
        for t_idx, xn, dest, swap in (
            (2, "xkd", kcat, False), (3, "xkt", kcat, True),
            (0, "xqd", qcat, False), (1, "xqt", qcat, True),
        ):
            for P in range(2):
                col = t_idx * 2 + P
                for chp in range(2):
                    pss = [(2 * chp, qkps.tile([128, 512], F32, tag="qkpA",
                                               name="qkpA")),
                           (2 * chp + 1, qkps.tile([128, 512], F32,
                                                   tag="qkpB", name="qkpB"))]
                    for j in range(2):
                        for cc, ps in pss:
                            nc.tensor.matmul(
                                ps[:], lhsT=wqk_sb[:, col, j, :, :],
                                rhs=xt[xn][:, 2 * j:2 * j + 2,
                                           cc * 512:(cc + 1) * 512],
                                start=(j == 0), stop=(j == 1), perf_mode=DR)
                    h_lo = 2 * P + (1 if swap else 0)
                    h_hi = 2 * P + (0 if swap else 1)
                    for cc, ps in pss:
                        cs = slice(cc * 512, (cc + 1) * 512)
                        if cc % 2 == 0:
                            nc.vector.tensor_scalar(
                                dest[h_lo][0:64, cs], ps[0:64, :],
                                1.0 / 16.0, bqk_sb[0:64, col:col + 1],
                                op0=Alu.mult, op1=Alu.add)
                            nc.vector.tensor_scalar(
                                dest[h_hi][64:128, cs], ps[64:128, :],
                                1.0 / 16.0, bqk_sb[64:128, col:col + 1],
                                op0=Alu.mult, op1=Alu.add)
                        else:
                            nc.scalar.activation(
                                dest[h_lo][0:64, cs], ps[0:64, :],
                                Act.Identity,
                                bias=bqk_sb[0:64, col:col + 1],
                                scale=1.0 / 16.0)
                            nc.scalar.activation(
                                dest[h_hi][64:128, cs], ps[64:128, :],
                                Act.Identity,
                                bias=bqk_sb[64:128, col:col + 1],
                                scale=1.0 / 16.0)

        # V: stationary xT (t-major psum), hi+lo weights, summed streams.
        # Loop order keeps matmuls sharing an xT stationary slice adjacent
        # (hi+lo pairs) so their LDWEIGHTS dedupe.
        bv_hd = bvB[:, :].rearrange("p (h c) -> p h c", c=64)
        for sc_i in range(16):
            ps = vps.tile([128, 256], F32, tag="vp", name="vp")
            if V_HILO:
                seq = [(0, "xvd", 0), (1, "xvd", 0), (0, "xvd", 1),
                       (1, "xvd", 1), (2, "xvt", 0), (3, "xvt", 0),
                       (2, "xvt", 1), (3, "xvt", 1)]
            else:
                seq = [(0, "xvd", 0), (0, "xvd", 1),
                       (1, "xvt", 0), (1, "xvt", 1)]
            nk = len(seq)
            for k, (widx, xn, j) in enumerate(seq):
                nc.tensor.matmul(
                    ps[:],
                    lhsT=xt[xn][:, 2 * j:2 * j + 2,
                                sc_i * 128:(sc_i + 1) * 128],
                    rhs=wv_sb[:, widx, j, :, :],
                    start=(k == 0), stop=(k == nk - 1), perf_mode=DR)
            psv = ps[:, :].rearrange("p (h c) -> p h c", c=64)
            nc.vector.scalar_tensor_tensor(
                out=vsum[:, sc_i, :, 0:64], in0=psv[:], scalar=VSC,
                in1=bv_hd[:], op0=Alu.mult, op1=Alu.add)

    if phases == "proj":
        nc.sync.dma_start(d_out.ap()[0:128, :], resb[:, 0, :])
        return
    # ---- Phase B: attention (software-pipelined) ------------------------
    # Per 128-key chunk: 2 score matmuls -> exp split across ACT (query
    # cols 0:512, exact exp to f8e5) and DVE (cols 512:1024, Schraudolph
    # affine written straight into a second f8e5 tile as uint8 bits -- no
    # convert pass, and no shared-tile write dep between the two engines).
    # PV runs two chunk-pairs behind the scores. Softmax-sum reciprocal is
    # DMA-reshaped [1,1024]->[128,8] so the DVE divide uses all lanes.
    with (
        tc.tile_pool(name="sc_ps", bufs=2, space="PSUM") as scp,
        tc.tile_pool(name="ctx_ps", bufs=2, space="PSUM") as ctxp,
        tc.tile_pool(name="at", bufs=3) as atp,
        tc.tile_pool(name="rin", bufs=2) as rip,
        tc.tile_pool(name="rb", bufs=2) as rbp,
        tc.tile_pool(name="ctmp", bufs=2) as ctmp,
    ):
        def emit_pv(ctx_ps, h, at2, c2):
            # NOTE: start=True zeroes the whole 2KB PSUM bank, so only
            # the first chunk touching each bank may start.
            for o in range(2):
                nc.tensor.matmul(
                    ctx_ps[0:68, o * 512:(o + 1) * 512],
                    lhsT=vsum[:, 2 * c2:2 * c2 + 2, h, 0:68],
                    rhs=at2[:, :, o * 512:(o + 1) * 512],
                    start=(c2 == 0), stop=(c2 == 7), perf_mode=DR)

        # evac: divide rows 0:64 by sum row 64 -> ctp pair tile. Staged so
        # each op is EMITTED into its engine queue a few chunks into the
        # NEXT (h,qh) stream -- by then its deps are met and the strict
        # FIFO engine queues don't stall the per-chunk exp work behind it.
        # Even heads land partition-aligned; odd heads go through a temp
        # tile + partition-shifted SBUF DMA (engines can't shift).
        def evac_stage1(ctx_ps, h, qh):
            rin = rip.tile([1, 1024], F32, tag="rin", name="rin")
            nc.vector.tensor_copy(rin[:], ctx_ps[64:65, :])
            r128 = rip.tile([128, 8], F32, tag="r128", name="r128")
            nc.sync.dma_start(r128[:], rin[:])
            r128b = rip.tile([128, 8], F32, tag="r128b", name="r128b")
            nc.vector.reciprocal(r128b[:], r128[:])
            rin2 = rip.tile([1, 1024], F32, tag="rin2", name="rin2")
            nc.sync.dma_start(rin2[:], r128b[:])
            return rin2

        def evac_stage2(ctx_ps, h, qh, rin2):
            par, P = h % 2, h // 2
            rb = rbp.tile([64, 1024], F32, tag="rb", name="rb")
            nc.gpsimd.partition_broadcast(rb[:], rin2[:])
            if par == 0:
                nc.vector.tensor_mul(
                    ctp[P][0:64, qh * 1024:(qh + 1) * 1024],
                    ctx_ps[0:64, :], rb[:])
            else:
                ct = ctmp.tile([64, 1024], BF, tag="ct", name="ct")
                nc.vector.tensor_mul(ct[:], ctx_ps[0:64, :], rb[:])
                nc.sync.dma_start(
                    ctp[P][64:128, qh * 1024:(qh + 1) * 1024], ct[:])

        pend_evac = None   # (ctx_ps, h, qh, rin2_or_None)
        for h in range(HL):
            for qh in range(2):
                ctx_ps = ctxp.tile([128, 1024], F32, tag="ctx", name="ctx")
                pendq = []   # PV runs TWO chunk-pairs behind the scores so
                # it never waits on the exp writes of the at2 it consumes
                for c2 in range(8):
                    # exp engine alternates by c2: even c2 -> ACT (exact
                    # exp), odd c2 -> DVE (Schraudolph-to-u8). One full
                    # [128,1024] op per key-chunk halves the per-op
                    # dispatch/drain overhead vs column-splitting, and
                    # each c2's chunk pair stays in one tile so the PV
                    # DoubleRow pairing is unchanged.
                    use_act = (c2 % 2 == 0)
                    at2 = atp.tile([128, 2, 1024], AT_DT,
                                   tag="atA" if use_act else "atB",
                                   name="atA" if use_act else "atB")
                    for i in range(2):
                        tcn = 2 * c2 + i
                        sc = scp.tile([128, 1024], F32, tag="sc", name="sc")
                        for half in range(2):
                            nc.tensor.matmul(
                                sc[:, half * 512:(half + 1) * 512],
                                lhsT=kcat[h][:, tcn * 128:(tcn + 1) * 128],
                                rhs=qcat[h][:, qh * 1024 + half * 512:
                                            qh * 1024 + (half + 1) * 512],
                                start=True, stop=True)
                        if use_act:
                            nc.scalar.activation(
                                at2[:, i, :], sc[:], Act.Exp,
                                bias=negC[:, 0:1], scale=0.125)
                        else:
                            nc.vector.tensor_scalar(
                                at2[:, i, :].bitcast(U8), sc[:],
                                SCHR8_A, B8, op0=Alu.mult, op1=Alu.add)
                    pendq.append((ctx_ps, h, at2, c2))
                    if len(pendq) == 3:
                        emit_pv(*pendq.pop(0))
                    if c2 == 1 and pend_evac is not None:
                        pend_evac = pend_evac[:3] + (
                            evac_stage1(*pend_evac[:3]),)
                    elif c2 == 4 and pend_evac is not None:
                        evac_stage2(*pend_evac)
                        pend_evac = None
                for pq in pendq:
                    emit_pv(*pq)
                pendq = []
                pend_evac = (ctx_ps, h, qh, None)
        pend_evac = pend_evac[:3] + (evac_stage1(*pend_evac[:3]),)
        evac_stage2(*pend_evac)

    if phases == "projattn":
        nc.sync.dma_start(d_out.ap()[0:128, :], resb[:, 0, :])
        return
    # ---- Phase C: out-proj partial + ReduceScatter + layernorm ----------
    with (
        tc.tile_pool(name="dram", bufs=1, space="DRAM") as dram,
        tc.tile_pool(name="o_ps", bufs=2, space="PSUM") as ops,
        tc.tile_pool(name="ob", bufs=3) as obp,
        tc.tile_pool(name="xs", bufs=2) as xsp,
        tc.tile_pool(name="ss", bufs=2) as ssp,
    ):
        snd = dram.tile([S, D], BF, tag="snd")
        rcv = dram.tile([SQ, D], BF, tag="rcv")
        for st in range(16):
            po = ops.tile([128, D], F32, tag="po", name="po")
            for P in range(2):
                nc.tensor.matmul(
                    po[:], lhsT=ctp[P][:, st * 128:(st + 1) * 128],
                    rhs=wo_sb[:, P, :], start=(P == 0), stop=(P == 1))
            ob = obp.tile([128, D], BF, tag="ob", name="ob")
            nc.scalar.copy(ob[:], po[:])
            eng = nc.sync if st % 2 == 0 else nc.scalar
            eng.dma_start(snd[st * 128:(st + 1) * 128, :], ob[:])
        if do_rs:
            nc.gpsimd.collective_compute(
                "ReduceScatter", Alu.add, replica_groups=RG_PAIRS,
                ins=[snd.opt()], outs=[rcv.opt()])
            rsx = int(os.environ.get("RSX", "0"))
            if rsx:
                dum = [dram.tile([SQ, D], BF, tag=f"dum{i}", name=f"dum{i}")
                       for i in range(2)]
                for i in range(rsx):
                    nc.gpsimd.collective_compute(
                        "ReduceScatter", Alu.add, replica_groups=RG_PAIRS,
                        ins=[snd.opt()], outs=[dum[i % 2].opt()])
        # layernorm: two passes. Pass 1 accumulates x = rcv + residual
        # in-place over resb and collects bn-stats; a single batched
        # Ln+Exp pair (one table set switch each, not per-st) computes
        # rs = (var+eps)^-0.5 for all 8 st at once; pass 2 is one fused
        # (x*rs - mu*rs) op per st. gamma/beta are identity here (host
        # applies them if ever nontrivial).
        mv8 = ssp.tile([128, 8, 2], F32, tag="mv8", name="mv8")
        for st in range(8):
            xr = obp.tile([128, D], BF, tag="xr", name="xr")
            eng = nc.sync if st % 2 == 0 else nc.scalar
            eng.dma_start(xr[:], rcv[st * 128:(st + 1) * 128, :])
            nc.vector.tensor_add(resb[:, st, :], xr[:], resb[:, st, :])
            bst = ssp.tile([128, 6], F32, tag="bst", name="bst")
            nc.vector.bn_stats(bst[:], resb[:, st, :])
            nc.vector.bn_aggr(mv8[:, st, :], bst[:])
        lnv8 = ssp.tile([128, 8], F32, tag="lnv8", name="lnv8")
        nc.scalar.activation(lnv8[:], mv8[:, :, 1], Act.Ln, bias=epsC[:, 0:1])
        rs8 = ssp.tile([128, 8], F32, tag="rs8", name="rs8")
        nc.scalar.activation(rs8[:], lnv8[:], Act.Exp, scale=mhalfC[:, 0:1])
        nmr8 = ssp.tile([128, 8], F32, tag="nmr8", name="nmr8")
        nc.vector.scalar_tensor_tensor(
            out=nmr8[:], in0=mv8[:, :, 0], scalar=-1.0, in1=rs8[:],
            op0=Alu.mult, op1=Alu.mult)
        for st in range(8):
            y = xsp.tile([128, D], F32, tag="y", name="y")
            nc.vector.tensor_scalar(
                y[:], resb[:, st, :], rs8[:, st:st + 1],
                nmr8[:, st:st + 1], op0=Alu.mult, op1=Alu.add)
            eng = nc.sync if st % 2 == 1 else nc.scalar
            eng.dma_start(d_out.ap()[st * 128:(st + 1) * 128, :], y[:])


def get_module(reps=1):
    phases = os.environ.get("KPHASES", "all")
    key = (reps, phases)
    if key not in _MODULES:
        _MODULES[key] = _build_module(reps, phases)
    return _MODULES[key]


def make_in_maps(inputs):
    """Build the 8 per-core input maps from the full problem inputs."""
    xT = {}
    for key in ("Q_data", "Q_time", "K_data", "K_time", "V_data", "V_time"):
        xT[key] = [np.ascontiguousarray(
            inputs[key][bb].astype(np.float32).T).astype(F8E4_NP)
            for bb in range(B)]

    F8V_NP = F8E4_NP if V_E4 else F8E5_NP
    VBS = 16.0 if V_E4 else 1.0

    def wq_tile(Wfull, heads, swap):
        # [512, 128] cols = [head a | head b] (b,a if swap); -> [128,2,2,128]
        a, b = heads
        if swap:
            a, b = b, a
        cols = np.concatenate(
            [Wfull[:, a * DH:(a + 1) * DH], Wfull[:, b * DH:(b + 1) * DH]], 1)
        w = (16.0 * cols).astype(F8E4_NP)
        return np.ascontiguousarray(
            w.reshape(2, 2, 128, 128).transpose(2, 0, 1, 3))

    def bq_col(bfull, heads, swap):
        a, b = heads
        if swap:
            a, b = b, a
        return np.concatenate(
            [bfull[a * DH:(a + 1) * DH], bfull[b * DH:(b + 1) * DH]]).astype(
                np.float32)

    in_maps = []
    for c in range(8):
        bb, g = divmod(c, 2)
        h0 = 4 * g
        m = {
            "xqd": xT["Q_data"][bb], "xqt": xT["Q_time"][bb],
            "xkd": xT["K_data"][bb], "xkt": xT["K_time"][bb],
            "xvd": xT["V_data"][bb], "xvt": xT["V_time"][bb],
        }
        # wqk [8, 128, 2, 2, 128]: idx = t*2 + P; t order qd,qt,kd,kt
        wqk = np.empty((8, 128, 2, 2, 128), F8E4_NP)
        bqk = np.empty((128, 8), np.float32)
        for t_i, (wkey, bkey, swap) in enumerate((
                ("Wq_d", "bq_d", False), ("Wq_t", "bq_t", True),
                ("Wk_d", "bk_d", False), ("Wk_t", "bk_t", True))):
            W = np.asarray(inputs[wkey], np.float32)
            bias = np.asarray(inputs[bkey], np.float32)
            for P in range(2):
                heads = (h0 + 2 * P, h0 + 2 * P + 1)
                wqk[t_i * 2 + P] = wq_tile(W, heads, swap)
                bqk[:, t_i * 2 + P] = bq_col(bias, heads, swap)
        m["wqk"] = wqk
        m["bqk"] = bqk
        # wv: vd_hi, vd_lo, vt_hi, vt_lo (or hi-only) in natural cols
        wv = np.empty((4 if V_HILO else 2, 128, 2, 2, 256), F8E4_NP)
        for s_i, wkey in enumerate(("Wv_d", "Wv_t")):
            W = 16.0 * np.asarray(
                inputs[wkey], np.float32)[:, h0 * DH:(h0 + 4) * DH]
            hi = W.astype(F8E4_NP)
            if V_HILO:
                lo = (W - hi.astype(np.float32)).astype(F8E4_NP)
                for k_i, Wp in enumerate((hi, lo)):
                    wv[s_i * 2 + k_i] = np.ascontiguousarray(
                        Wp.reshape(2, 2, 128, 256).transpose(2, 0, 1, 3))
            else:
                wv[s_i] = np.ascontiguousarray(
                    hi.reshape(2, 2, 128, 256).transpose(2, 0, 1, 3))
        m["wv"] = wv
        bv = (np.asarray(inputs["bv_d"], np.float32)
              + np.asarray(inputs["bv_t"], np.float32))[
                  h0 * DH:(h0 + 4) * DH] * VBS
        m["bv"] = bv.reshape(1, 256)
        # wo [2, 128, 512]: pair P rows = heads (h0+2P, h0+2P+1) dims
        wo = np.empty((2, 128, D), BF_NP)
        Wo = np.asarray(inputs["Wo"], np.float32)
        for P in range(2):
            r0 = (h0 + 2 * P) * DH
            wo[P] = Wo[r0:r0 + 128, :].astype(BF_NP)
        m["wo"] = wo
        m["qres"] = np.ascontiguousarray(
            inputs["Q_data"][bb, g * SQ:(g + 1) * SQ, :].astype(BF_NP))
        m["bo"] = np.asarray(inputs["bo"], np.float32).reshape(1, D)

        in_maps.append(m)
    return in_maps


def kernel(**inputs):
    inputs = {k: np.asarray(v) for k, v in inputs.items()}
    nc = get_module(reps=1)
    in_maps = make_in_maps(inputs)
    res = bass_utils.run_bass_kernel_spmd(nc, in_maps, core_ids=list(range(8)))
    out = np.empty((B, S, D), dtype=np.float32)
    for c in range(8):
        bb, g = divmod(c, 2)
        out[bb, g * SQ:(g + 1) * SQ, :] = res.results[c]["out"]
    # gamma/beta are identity in this problem; apply on host if not.
    gamma = np.asarray(inputs["gamma"], np.float32)
    beta = np.asarray(inputs["beta"], np.float32)
    if np.any(gamma != 1.0):
        out *= gamma
    if np.any(beta != 0.0):
        out += beta
    return out

